# revision 5
# baseline (speedup 1.0000x reference)
"""Multi-head self-attention Trainium2 kernel (8 NeuronCores, SPMD).

Problem: X[2,2048,1024] @ {Wq,Wk,Wv}[1024,1024] (+bias), 16 heads x 64 dim,
scores/Dh^2 softmax (no max-subtraction needed: logits are tiny), attn @ V.

Sharding: 8 cores = 2 batches x 4 head-groups (4 heads each). Each core
computes its batch's QKV projection restricted to its 256 output columns,
then full attention for its 4 heads over the whole sequence. Per-core
output is out[b, :, hg*256:(hg+1)*256] — gathered host-side.

Core dataflow (per core):
  phase B: X[b] -> SBUF, PE-transpose to X^T [1024, 2048] (d_in on partitions)
  phase C: Q^T,K^T [256,2048] = W.T @ X^T (+bias via rank-1 ones matmul),
           V' [2048, 4x65] = X @ Wv (+bias), with a ones column per head
           (denominator trick for the softmax).
  phase D: per (q-chunk, head): S^T[k,q] = K^T.T @ Q^T, exp on ACT
           (scale=1/4096 folded in), out'^T[65,q] += V'.T @ E^T accumulated
           over k-tiles; epilogue transposes out'^T and divides by the
           ones-row (softmax denominator).
All matmuls run as float32r (1 cycle/row at N>=256 vs 4 for fp32).
"""

import os
import numpy as np

S = 2048
D_IN = 1024
D_OUT = 1024
N_HEADS = 16
DH = 64
N_CORES = 8
HG = 4            # heads per core
CS = HG * DH      # 256 output cols per core
KT_N = S // 128   # 16 k-tiles
QC_N = 4          # q-chunks of 512
QCW = 512

F32R_MM = True    # matmul operand dtype: float32r (fast) vs float32 (4x slower)

_COMPILED = {}


def _build_program():
    import concourse.bass as bass
    import concourse.tile as tile
    from concourse import bacc, mybir
    from concourse.masks import make_identity

    f32 = mybir.dt.float32
    f32r = mybir.dt.float32r if F32R_MM else mybir.dt.float32
    Exp = mybir.ActivationFunctionType.Exp

    nc = bacc.Bacc(
        "TRN2",
        target_bir_lowering=False,
        debug=False,
        enable_asserts=False,
        num_devices=N_CORES,
    )

    X = nc.dram_tensor("X_s", [S, D_IN], f32, kind="ExternalInput").ap()
    Wq = nc.dram_tensor("Wq_s", [D_IN, CS], f32, kind="ExternalInput").ap()
    Wk = nc.dram_tensor("Wk_s", [D_IN, CS], f32, kind="ExternalInput").ap()
    Wv = nc.dram_tensor("Wv_s", [D_IN, CS], f32, kind="ExternalInput").ap()
    bq = nc.dram_tensor("bq_s", [1, CS], f32, kind="ExternalInput").ap()
    bk = nc.dram_tensor("bk_s", [1, CS], f32, kind="ExternalInput").ap()
    bv = nc.dram_tensor("bv_s", [1, CS], f32, kind="ExternalInput").ap()
    O = nc.dram_tensor("O_s", [S, CS], f32, kind="ExternalOutput").ap()

    KD_N = D_IN // 128  # 8 d_in tiles
    ST_N = S // 128     # 16 s-tiles

    with tile.TileContext(nc) as tc:
        with (
            tc.tile_pool(name="persist", bufs=1) as persist,
            tc.tile_pool(name="consts", bufs=1) as consts,
        ):
            ident = consts.tile([128, 128], f32)
            make_identity(nc, ident)
            # all matmul operands must be float32r-rounded by their writer
            # (and memset can't write f32r, so memset f32 then copy-round)
            ones_f32 = consts.tile([1, QCW], f32)
            nc.vector.memset(ones_f32, 1.0)
            ones_row = consts.tile([1, QCW], f32r)
            nc.vector.tensor_copy(ones_row, ones_f32)
            ones4 = consts.tile([128, HG], f32)
            nc.vector.memset(ones4, 1.0)

            b_stage = consts.tile([1, 3, CS], f32)
            nc.sync.dma_start(out=b_stage[:, 0, :], in_=bq)
            nc.sync.dma_start(out=b_stage[:, 1, :], in_=bk)
            nc.sync.dma_start(out=b_stage[:, 2, :], in_=bv)
            b_rows = consts.tile([1, 3, CS], f32r)
            nc.vector.tensor_copy(b_rows, b_stage)

            # big persistent SBUF tensors
            xt_all = persist.tile([128, KD_N, S], f32r)       # X^T   64KB/part
            qt_all = persist.tile([128, 2, S], f32r)          # Q^T   16KB/part
            kt_all = persist.tile([128, 2, S], f32r)          # K^T   16KB/part
            vp_all = persist.tile([128, ST_N, HG, DH + 1], f32r)  # V' 16.25KB/part
            w_all = persist.tile([128, 3, KD_N, CS], f32r)    # Wq,Wk,Wv 24KB/part

            with tc.tile_pool(name="wload", bufs=4) as wload:
                for wi, wap in enumerate((Wq, Wk, Wv)):
                    for kd in range(KD_N):
                        wst = wload.tile([128, CS], f32)
                        nc.sync.dma_start(
                            out=wst, in_=wap[kd * 128:(kd + 1) * 128, :]
                        )
                        dst = w_all[:, wi, kd, :]
                        if (wi * KD_N + kd) % 2 == 0:
                            nc.vector.tensor_copy(dst, wst)
                        else:
                            nc.scalar.copy(dst, wst)

            # ones column for the softmax-denominator trick
            for st in range(ST_N):
                nc.vector.tensor_copy(vp_all[:, st, :, DH:DH + 1], ones4)

            # ---------------- phase B: X^T ----------------
            with (
                tc.tile_pool(name="xload", bufs=3) as xload,
                tc.tile_pool(name="tp", bufs=4, space="PSUM") as tp,
            ):
                for st in range(ST_N):
                    x_tile = xload.tile([128, D_IN], f32)
                    nc.sync.dma_start(
                        out=x_tile, in_=X[st * 128:(st + 1) * 128, :]
                    )
                    for kd in range(KD_N):
                        pt = tp.tile([128, 128], f32, space="PSUM")
                        nc.tensor.transpose(
                            pt, x_tile[:, kd * 128:(kd + 1) * 128], ident
                        )
                        dst = xt_all[:, kd, st * 128:(st + 1) * 128]
                        if (st * KD_N + kd) % 2 == 0:
                            nc.vector.tensor_copy(dst, pt)
                        else:
                            nc.scalar.copy(dst, pt)

            # ---------------- phase C: projections ----------------
            with tc.tile_pool(name="pqk", bufs=3, space="PSUM") as pqk:
                for wi, dst_all in ((0, qt_all), (1, kt_all)):
                    for m in range(2):
                        for n_ in range(QC_N):
                            ps = pqk.tile([128, QCW], f32, space="PSUM")
                            for kd in range(KD_N):
                                nc.tensor.matmul(
                                    ps,
                                    lhsT=(w_all[:, wi, kd, m * 128:(m + 1) * 128]),
                                    rhs=(xt_all[:, kd, n_ * QCW:(n_ + 1) * QCW]),
                                    start=(kd == 0),
                                    stop=False,
                                )
                            nc.tensor.matmul(
                                ps,
                                lhsT=(b_rows[:, wi, m * 128:(m + 1) * 128]),
                                rhs=(ones_row),
                                start=False,
                                stop=True,
                            )
                            nc.scalar.copy(
                                dst_all[:, m, n_ * QCW:(n_ + 1) * QCW], ps
                            )

                for st in range(ST_N):
                    ps = pqk.tile([128, CS], f32, space="PSUM", tag="psv")
                    for kd in range(KD_N):
                        nc.tensor.matmul(
                            ps,
                            lhsT=(xt_all[:, kd, st * 128:(st + 1) * 128]),
                            rhs=(w_all[:, 2, kd, :]),
                            start=(kd == 0),
                            stop=False,
                        )
                    nc.tensor.matmul(
                        ps,
                        lhsT=(ones_row[:, 0:128]),
                        rhs=(b_rows[:, 2, :]),
                        start=False,
                        stop=True,
                    )
                    for j in range(HG):
                        nc.vector.tensor_copy(
                            vp_all[:, st, j, 0:DH], ps[:, j * DH:(j + 1) * DH]
                        )

            # ---------------- phase D: attention ----------------
            with (
                tc.tile_pool(name="sps", bufs=3, space="PSUM") as spsp,
                tc.tile_pool(name="acc", bufs=2, space="PSUM") as accp,
                tc.tile_pool(name="tp2", bufs=2, space="PSUM") as tp2,
                tc.tile_pool(name="et", bufs=4) as etp,
                tc.tile_pool(name="ot", bufs=2) as otp,
                tc.tile_pool(name="eps", bufs=4) as epp,
                tc.tile_pool(name="stage", bufs=2) as stagep,
            ):
                for qc in range(QC_N):
                    stage = stagep.tile([128, 4, CS], f32)
                    for j in range(HG):
                        po = (j % 2) * 64
                        mt = j // 2
                        acc = accp.tile([DH + 1, QCW], f32, space="PSUM")
                        ets = []
                        # software-pipelined: matmul1[kt+1] issues before
                        # matmul2[kt] so PE never stalls on ACT's exp
                        for kt in range(KT_N):
                            sps = spsp.tile([128, QCW], f32, space="PSUM")
                            nc.tensor.matmul(
                                sps,
                                lhsT=(kt_all[po:po + 64, mt, kt * 128:(kt + 1) * 128]),
                                rhs=(qt_all[po:po + 64, mt, qc * QCW:(qc + 1) * QCW]),
                                start=True,
                                stop=True,
                            )
                            et = etp.tile([128, QCW], f32r)
                            nc.scalar.activation(et, sps, Exp, scale=1.0 / 4096.0)
                            ets.append(et)
                            if kt > 0:
                                nc.tensor.matmul(
                                    acc,
                                    lhsT=(vp_all[:, kt - 1, j, :]),
                                    rhs=(ets[kt - 1]),
                                    start=(kt - 1 == 0),
                                    stop=False,
                                )
                        nc.tensor.matmul(
                            acc,
                            lhsT=(vp_all[:, KT_N - 1, j, :]),
                            rhs=(ets[KT_N - 1]),
                            start=False,
                            stop=True,
                        )
                        # epilogue: transpose [65, 512] -> 4x [128, 65],
                        # divide by the denominator row, stage for DMA
                        ot = otp.tile([DH + 1, QCW], f32)
                        nc.vector.tensor_copy(ot, acc)
                        for tj in range(4):
                            pt = tp2.tile([128, DH + 1], f32, space="PSUM")
                            nc.tensor.transpose(
                                pt, ot[:, tj * 128:(tj + 1) * 128],
                                ident[0:DH + 1, 0:DH + 1],
                            )
                            rcp = epp.tile([128, 1], f32)
                            nc.vector.reciprocal(rcp, pt[:, DH:DH + 1])
                            nc.vector.tensor_scalar_mul(
                                stage[:, tj, j * DH:(j + 1) * DH],
                                pt[:, 0:DH],
                                rcp,
                            )
                    for tj in range(4):
                        nc.sync.dma_start(
                            out=O[qc * QCW + tj * 128: qc * QCW + (tj + 1) * 128, :],
                            in_=stage[:, tj, :],
                        )

    nc.compile()
    return nc


def _get_program():
    if "nc" not in _COMPILED:
        _COMPILED["nc"] = _build_program()
    return _COMPILED["nc"]


def make_in_maps(X, Wq, bq, Wk, bk, Wv, bv):
    in_maps = []
    for core in range(N_CORES):
        b, hg = core // HG, core % HG
        cs = slice(hg * CS, (hg + 1) * CS)
        in_maps.append({
            "X_s": np.ascontiguousarray(X[b]),
            "Wq_s": np.ascontiguousarray(Wq[:, cs]),
            "Wk_s": np.ascontiguousarray(Wk[:, cs]),
            "Wv_s": np.ascontiguousarray(Wv[:, cs]),
            "bq_s": np.ascontiguousarray(bq[cs]).reshape(1, CS),
            "bk_s": np.ascontiguousarray(bk[cs]).reshape(1, CS),
            "bv_s": np.ascontiguousarray(bv[cs]).reshape(1, CS),
        })
    return in_maps


def gather_output(results):
    out = np.zeros((2, S, D_OUT), np.float32)
    for core in range(N_CORES):
        b, hg = core // HG, core % HG
        out[b, :, hg * CS:(hg + 1) * CS] = results[core]["O_s"]
    return out


def run(X, Wq, bq, Wk, bk, Wv, bv, trace=False, tmpdir=None):
    from concourse import bass_utils

    nc = _get_program()
    in_maps = make_in_maps(X, Wq, bq, Wk, bk, Wv, bv)
    res = bass_utils.run_bass_kernel_spmd(
        nc, in_maps, core_ids=list(range(N_CORES)), trace=trace, tmpdir=tmpdir,
    )
    return gather_output(res.results), res


def kernel(X, Wq, bq, Wk, bk, Wv, bv):
    out, _ = run(
        np.asarray(X), np.asarray(Wq), np.asarray(bq), np.asarray(Wk),
        np.asarray(bk), np.asarray(Wv), np.asarray(bv),
    )
    return out


if __name__ == "__main__":
    rng = np.random.default_rng(0)
    inputs = {
        "X": rng.standard_normal((2, S, D_IN), dtype=np.float32),
        "Wq": rng.standard_normal((D_IN, D_OUT), dtype=np.float32) / 32,
        "bq": rng.standard_normal(D_OUT, dtype=np.float32) * 0.01,
        "Wk": rng.standard_normal((D_IN, D_OUT), dtype=np.float32) / 32,
        "bk": rng.standard_normal(D_OUT, dtype=np.float32) * 0.01,
        "Wv": rng.standard_normal((D_IN, D_OUT), dtype=np.float32) / 32,
        "bv": rng.standard_normal(D_OUT, dtype=np.float32) * 0.01,
    }
    out = kernel(**inputs)
    print("out", out.shape, out.dtype, np.abs(out).max())


# revision 13
# speedup vs baseline: 1.1241x; 1.1241x over previous
"""Multi-head self-attention Trainium2 kernel (8 NeuronCores, SPMD).

Problem: X[2,2048,1024] @ {Wq,Wk,Wv}[1024,1024] (+bias), 16 heads x 64 dim,
scores/Dh^2 softmax (no max-subtraction needed: logits are tiny), attn @ V.

Sharding: 8 cores = 2 batches x 4 head-groups (4 heads each). Each core
computes its batch's QKV projection restricted to its 256 output columns,
then full attention for its 4 heads over the whole sequence. Per-core
output is out[b, :, hg*256:(hg+1)*256] — gathered host-side.

Core dataflow (per core):
  phase B: X[b] -> SBUF, PE-transpose to X^T [1024, 2048] (d_in on partitions)
  phase C: Q^T,K^T [256,2048] = W.T @ X^T (+bias via rank-1 ones matmul),
           V' [2048, 4x65] = X @ Wv (+bias), with a ones column per head
           (denominator trick for the softmax).
  phase D: per (q-chunk, head): S^T[k,q] = K^T.T @ Q^T, exp on ACT
           (scale=1/4096 folded in), out'^T[65,q] += V'.T @ E^T accumulated
           over k-tiles; epilogue transposes out'^T and divides by the
           ones-row (softmax denominator).
All matmuls run as float32r (1 cycle/row at N>=256 vs 4 for fp32).
"""

import os
import numpy as np

S = 2048
D_IN = 1024
D_OUT = 1024
N_HEADS = 16
DH = 64
N_CORES = 8
HG = 4            # heads per core
CS = HG * DH      # 256 output cols per core
KT_N = S // 128   # 16 k-tiles
QC_N = 4          # q-chunks of 512
QCW = 512

F32R_MM = True    # proj matmul operand dtype: float32r (fast) vs float32 (4x slower)
BF16_ATTN = True  # attention chains (Q/K/V'/E) in bf16: lower PE power (dodges
                  # the all-core power throttle), FWL weight loads; ~1e-3 rel err
EXP_BATCH = 1     # kt tiles per ACTIVATE (bisect: 1 = per-bank exp)

_COMPILED = {}


def _build_program():
    import concourse.bass as bass
    import concourse.tile as tile
    from concourse import bacc, mybir
    from concourse.masks import make_identity

    f32 = mybir.dt.float32
    f32r = mybir.dt.float32r if F32R_MM else mybir.dt.float32
    bf16 = mybir.dt.bfloat16 if BF16_ATTN else f32r
    Exp = mybir.ActivationFunctionType.Exp

    nc = bacc.Bacc(
        "TRN2",
        target_bir_lowering=False,
        debug=False,
        enable_asserts=False,
        num_devices=N_CORES,
    )

    X = nc.dram_tensor("X_s", [S, D_IN], f32, kind="ExternalInput").ap()
    Wq = nc.dram_tensor("Wq_s", [D_IN, CS], f32, kind="ExternalInput").ap()
    Wk = nc.dram_tensor("Wk_s", [D_IN, CS], f32, kind="ExternalInput").ap()
    Wv = nc.dram_tensor("Wv_s", [D_IN, CS], f32, kind="ExternalInput").ap()
    bq = nc.dram_tensor("bq_s", [1, CS], f32, kind="ExternalInput").ap()
    bk = nc.dram_tensor("bk_s", [1, CS], f32, kind="ExternalInput").ap()
    bv = nc.dram_tensor("bv_s", [1, CS], f32, kind="ExternalInput").ap()
    O = nc.dram_tensor("O_s", [S, CS], f32, kind="ExternalOutput").ap()

    KD_N = D_IN // 128  # 8 d_in tiles
    ST_N = S // 128     # 16 s-tiles

    with tile.TileContext(nc) as tc:
        with (
            tc.tile_pool(name="persist", bufs=1) as persist,
            tc.tile_pool(name="consts", bufs=1) as consts,
        ):
            ident = consts.tile([128, 128], f32)
            make_identity(nc, ident)
            # all matmul operands must be float32r-rounded by their writer
            # (and memset can't write f32r, so memset f32 then copy-round)
            ones_f32 = consts.tile([1, QCW], f32)
            nc.vector.memset(ones_f32, 1.0)
            ones_row = consts.tile([1, QCW], f32r)
            nc.vector.tensor_copy(ones_row, ones_f32)
            ones4 = consts.tile([128, HG], f32)
            nc.vector.memset(ones4, 1.0)

            b_stage = consts.tile([1, 3, CS], f32)
            nc.sync.dma_start(out=b_stage[:, 0, :], in_=bq)
            nc.sync.dma_start(out=b_stage[:, 1, :], in_=bk)
            nc.sync.dma_start(out=b_stage[:, 2, :], in_=bv)
            b_rows = consts.tile([1, 3, CS], f32r)
            nc.vector.tensor_copy(b_rows, b_stage)

            # big persistent SBUF tensors
            xt_all = persist.tile([128, KD_N, S], f32r)       # X^T   64KB/part
            qt_all = persist.tile([128, 2, S], bf16)          # Q^T
            kt_all = persist.tile([128, 2, S], bf16)          # K^T
            vp_all = persist.tile([128, ST_N, HG, DH + 1], bf16)  # V'
            w_all = persist.tile([128, 3, KD_N, CS], f32r)    # Wq,Wk,Wv 24KB/part

            with tc.tile_pool(name="wload", bufs=4) as wload:
                for wi, wap in enumerate((Wq, Wk, Wv)):
                    for kd in range(KD_N):
                        wst = wload.tile([128, CS], f32)
                        nc.sync.dma_start(
                            out=wst, in_=wap[kd * 128:(kd + 1) * 128, :]
                        )
                        dst = w_all[:, wi, kd, :]
                        if (wi * KD_N + kd) % 2 == 0:
                            nc.vector.tensor_copy(dst, wst)
                        else:
                            nc.scalar.copy(dst, wst)

            # ones column for the softmax-denominator trick
            for st in range(ST_N):
                nc.vector.tensor_copy(vp_all[:, st, :, DH:DH + 1], ones4)

            # ---------------- phase B: X^T ----------------
            with (
                tc.tile_pool(name="xload", bufs=3) as xload,
                tc.tile_pool(name="tp", bufs=4, space="PSUM") as tp,
            ):
                for st in range(ST_N):
                    x_tile = xload.tile([128, D_IN], f32)
                    nc.sync.dma_start(
                        out=x_tile, in_=X[st * 128:(st + 1) * 128, :]
                    )
                    for kd in range(KD_N):
                        pt = tp.tile([128, 128], f32, space="PSUM")
                        nc.tensor.transpose(
                            pt, x_tile[:, kd * 128:(kd + 1) * 128], ident
                        )
                        dst = xt_all[:, kd, st * 128:(st + 1) * 128]
                        if (st * KD_N + kd) % 2 == 0:
                            nc.vector.tensor_copy(dst, pt)
                        else:
                            nc.scalar.copy(dst, pt)

            # ---------------- phase C: projections ----------------
            with tc.tile_pool(name="pqk", bufs=3, space="PSUM") as pqk:
                for wi, dst_all in ((0, qt_all), (1, kt_all)):
                    for m in range(2):
                        for n_ in range(QC_N):
                            ps = pqk.tile([128, QCW], f32, space="PSUM")
                            for kd in range(KD_N):
                                nc.tensor.matmul(
                                    ps,
                                    lhsT=(w_all[:, wi, kd, m * 128:(m + 1) * 128]),
                                    rhs=(xt_all[:, kd, n_ * QCW:(n_ + 1) * QCW]),
                                    start=(kd == 0),
                                    stop=False,
                                )
                            nc.tensor.matmul(
                                ps,
                                lhsT=(b_rows[:, wi, m * 128:(m + 1) * 128]),
                                rhs=(ones_row),
                                start=False,
                                stop=True,
                            )
                            nc.scalar.copy(
                                dst_all[:, m, n_ * QCW:(n_ + 1) * QCW], ps
                            )

                for st in range(ST_N):
                    ps = pqk.tile([128, CS], f32, space="PSUM", tag="psv")
                    for kd in range(KD_N):
                        nc.tensor.matmul(
                            ps,
                            lhsT=(xt_all[:, kd, st * 128:(st + 1) * 128]),
                            rhs=(w_all[:, 2, kd, :]),
                            start=(kd == 0),
                            stop=False,
                        )
                    nc.tensor.matmul(
                        ps,
                        lhsT=(ones_row[:, 0:128]),
                        rhs=(b_rows[:, 2, :]),
                        start=False,
                        stop=True,
                    )
                    for j in range(HG):
                        nc.vector.tensor_copy(
                            vp_all[:, st, j, 0:DH], ps[:, j * DH:(j + 1) * DH]
                        )

            # ---------------- phase D: attention ----------------
            with (
                tc.tile_pool(name="sps", bufs=2, space="PSUM") as spsp,
                tc.tile_pool(name="acc", bufs=2, space="PSUM") as accp,
                tc.tile_pool(name="tp2", bufs=2, space="PSUM") as tp2,
                tc.tile_pool(name="et", bufs=4) as etp,
                tc.tile_pool(name="ot", bufs=2) as otp,
                tc.tile_pool(name="eps", bufs=4) as epp,
                tc.tile_pool(name="stage", bufs=2) as stagep,
            ):
                for qc in range(QC_N):
                    stage = stagep.tile([128, 4, CS], f32)
                    for j in range(HG):
                        po = (j % 2) * 64
                        mt = j // 2
                        acc = accp.tile([DH + 1, QCW], f32, space="PSUM")
                        ets = []
                        # EB kt-tiles share a PSUM tile so exp runs as one
                        # [128,EB*512] ACTIVATE (amortizes ACT per-inst cost).
                        # Software-pipelined: group g's matmul1s issue before
                        # group (g-1)'s matmul2s so PE never stalls on ACT.
                        EB = EXP_BATCH
                        G = KT_N // EB
                        for g in range(G):
                            sps = spsp.tile([128, EB * QCW], f32, space="PSUM")
                            for h in range(EB):
                                kt = EB * g + h
                                nc.tensor.matmul(
                                    sps[:, h * QCW:(h + 1) * QCW],
                                    lhsT=(kt_all[po:po + 64, mt, kt * 128:(kt + 1) * 128]),
                                    rhs=(qt_all[po:po + 64, mt, qc * QCW:(qc + 1) * QCW]),
                                    start=True,
                                    stop=True,
                                )
                            et = etp.tile([128, EB * QCW], bf16)
                            nc.scalar.activation(et, sps, Exp, scale=1.0 / 4096.0)
                            ets.append(et)
                            if g > 0:
                                for h in range(EB):
                                    kt = EB * (g - 1) + h
                                    nc.tensor.matmul(
                                        acc,
                                        lhsT=(vp_all[:, kt, j, :]),
                                        rhs=(ets[g - 1][:, h * QCW:(h + 1) * QCW]),
                                        start=(kt == 0),
                                        stop=False,
                                    )
                        for h in range(EB):
                            kt = EB * (G - 1) + h
                            nc.tensor.matmul(
                                acc,
                                lhsT=(vp_all[:, kt, j, :]),
                                rhs=(ets[G - 1][:, h * QCW:(h + 1) * QCW]),
                                start=False,
                                stop=(h == EB - 1),
                            )
                        # epilogue: transpose [65, 512] -> 4x [128, 65],
                        # divide by the denominator row, stage for DMA
                        ot = otp.tile([DH + 1, QCW], f32)
                        nc.vector.tensor_copy(ot, acc)
                        for tj in range(4):
                            pt = tp2.tile([128, DH + 1], f32, space="PSUM")
                            nc.tensor.transpose(
                                pt, ot[:, tj * 128:(tj + 1) * 128],
                                ident[0:DH + 1, 0:DH + 1],
                            )
                            rcp = epp.tile([128, 1], f32)
                            nc.vector.reciprocal(rcp, pt[:, DH:DH + 1])
                            nc.vector.tensor_scalar_mul(
                                stage[:, tj, j * DH:(j + 1) * DH],
                                pt[:, 0:DH],
                                rcp,
                            )
                    for tj in range(4):
                        nc.sync.dma_start(
                            out=O[qc * QCW + tj * 128: qc * QCW + (tj + 1) * 128, :],
                            in_=stage[:, tj, :],
                        )

    nc.compile()
    return nc


def _get_program():
    if "nc" not in _COMPILED:
        _COMPILED["nc"] = _build_program()
    return _COMPILED["nc"]


def make_in_maps(X, Wq, bq, Wk, bk, Wv, bv):
    in_maps = []
    for core in range(N_CORES):
        b, hg = core // HG, core % HG
        cs = slice(hg * CS, (hg + 1) * CS)
        in_maps.append({
            "X_s": np.ascontiguousarray(X[b]),
            "Wq_s": np.ascontiguousarray(Wq[:, cs]),
            "Wk_s": np.ascontiguousarray(Wk[:, cs]),
            "Wv_s": np.ascontiguousarray(Wv[:, cs]),
            "bq_s": np.ascontiguousarray(bq[cs]).reshape(1, CS),
            "bk_s": np.ascontiguousarray(bk[cs]).reshape(1, CS),
            "bv_s": np.ascontiguousarray(bv[cs]).reshape(1, CS),
        })
    return in_maps


def gather_output(results):
    out = np.zeros((2, S, D_OUT), np.float32)
    for core in range(N_CORES):
        b, hg = core // HG, core % HG
        out[b, :, hg * CS:(hg + 1) * CS] = results[core]["O_s"]
    return out


def run(X, Wq, bq, Wk, bk, Wv, bv, trace=False, tmpdir=None):
    from concourse import bass_utils

    nc = _get_program()
    in_maps = make_in_maps(X, Wq, bq, Wk, bk, Wv, bv)
    res = bass_utils.run_bass_kernel_spmd(
        nc, in_maps, core_ids=list(range(N_CORES)), trace=trace, tmpdir=tmpdir,
    )
    return gather_output(res.results), res


def kernel(X, Wq, bq, Wk, bk, Wv, bv):
    out, _ = run(
        np.asarray(X), np.asarray(Wq), np.asarray(bq), np.asarray(Wk),
        np.asarray(bk), np.asarray(Wv), np.asarray(bv),
    )
    return out


if __name__ == "__main__":
    rng = np.random.default_rng(0)
    inputs = {
        "X": rng.standard_normal((2, S, D_IN), dtype=np.float32),
        "Wq": rng.standard_normal((D_IN, D_OUT), dtype=np.float32) / 32,
        "bq": rng.standard_normal(D_OUT, dtype=np.float32) * 0.01,
        "Wk": rng.standard_normal((D_IN, D_OUT), dtype=np.float32) / 32,
        "bk": rng.standard_normal(D_OUT, dtype=np.float32) * 0.01,
        "Wv": rng.standard_normal((D_IN, D_OUT), dtype=np.float32) / 32,
        "bv": rng.standard_normal(D_OUT, dtype=np.float32) * 0.01,
    }
    out = kernel(**inputs)
    print("out", out.shape, out.dtype, np.abs(out).max())


# revision 17
# speedup vs baseline: 1.2229x; 1.0879x over previous
"""Multi-head self-attention Trainium2 kernel (8 NeuronCores, SPMD).

Problem: X[2,2048,1024] @ {Wq,Wk,Wv}[1024,1024] (+bias), 16 heads x 64 dim,
scores/Dh^2 softmax (no max-subtraction needed: logits are tiny), attn @ V.

Sharding: 8 cores = 2 batches x 4 head-groups (4 heads each). Each core
computes its batch's QKV projection restricted to its 256 output columns,
then full attention for its 4 heads over the whole sequence. Per-core
output is out[b, :, hg*256:(hg+1)*256] — gathered host-side.

Core dataflow (per core):
  phase B: X[b] -> SBUF, PE-transpose to X^T [1024, 2048] (d_in on partitions)
  phase C: Q^T,K^T [256,2048] = W.T @ X^T (+bias via rank-1 ones matmul),
           V' [2048, 4x65] = X @ Wv (+bias), with a ones column per head
           (denominator trick for the softmax).
  phase D: per (q-chunk, head): S^T[k,q] = K^T.T @ Q^T, exp on ACT
           (scale=1/4096 folded in), out'^T[65,q] += V'.T @ E^T accumulated
           over k-tiles; epilogue transposes out'^T and divides by the
           ones-row (softmax denominator).
All matmuls run as float32r (1 cycle/row at N>=256 vs 4 for fp32).
"""

import os
import numpy as np

S = 2048
D_IN = 1024
D_OUT = 1024
N_HEADS = 16
DH = 64
N_CORES = 8
HG = 4            # heads per core
CS = HG * DH      # 256 output cols per core
KT_N = S // 128   # 16 k-tiles
QC_N = 4          # q-chunks of 512
QCW = 512

F32R_MM = True    # proj matmul operand dtype: float32r (fast) vs float32 (4x slower)
BF16_ATTN = True  # attention chains (Q/K/V'/E) in bf16: lower PE power (dodges
                  # the all-core power throttle), FWL weight loads; ~1e-3 rel err
EXP_BATCH = 1     # kt tiles per ACTIVATE (bisect: 1 = per-bank exp)

_COMPILED = {}


def _build_program():
    import concourse.bass as bass
    import concourse.tile as tile
    from concourse import bacc, mybir
    from concourse.masks import make_identity

    f32 = mybir.dt.float32
    f32r = mybir.dt.float32r if F32R_MM else mybir.dt.float32
    bf16 = mybir.dt.bfloat16 if BF16_ATTN else f32r
    Exp = mybir.ActivationFunctionType.Exp

    nc = bacc.Bacc(
        "TRN2",
        target_bir_lowering=False,
        debug=False,
        enable_asserts=False,
        num_devices=N_CORES,
    )

    X = nc.dram_tensor("X_s", [S, D_IN], f32, kind="ExternalInput").ap()
    Wq = nc.dram_tensor("Wq_s", [D_IN, CS], f32, kind="ExternalInput").ap()
    Wk = nc.dram_tensor("Wk_s", [D_IN, CS], f32, kind="ExternalInput").ap()
    Wv = nc.dram_tensor("Wv_s", [D_IN, CS], f32, kind="ExternalInput").ap()
    bq = nc.dram_tensor("bq_s", [1, CS], f32, kind="ExternalInput").ap()
    bk = nc.dram_tensor("bk_s", [1, CS], f32, kind="ExternalInput").ap()
    bv = nc.dram_tensor("bv_s", [1, CS], f32, kind="ExternalInput").ap()
    O = nc.dram_tensor("O_s", [S, CS], f32, kind="ExternalOutput").ap()

    KD_N = D_IN // 128  # 8 d_in tiles
    ST_N = S // 128     # 16 s-tiles

    with tile.TileContext(nc) as tc:
        with (
            tc.tile_pool(name="persist", bufs=1) as persist,
            tc.tile_pool(name="consts", bufs=1) as consts,
        ):
            ident = consts.tile([128, 128], f32)
            make_identity(nc, ident)
            # all matmul operands must be float32r-rounded by their writer
            # (and memset can't write f32r, so memset f32 then copy-round)
            ones_f32 = consts.tile([1, QCW], f32)
            nc.vector.memset(ones_f32, 1.0)
            ones_row = consts.tile([1, QCW], f32r)
            nc.vector.tensor_copy(ones_row, ones_f32)
            ones4 = consts.tile([128, HG], f32)
            nc.vector.memset(ones4, 1.0)

            # big persistent SBUF tensors
            xt_all = persist.tile([128, KD_N, S], f32r)       # X^T   64KB/part
            qt_all = persist.tile([128, 2, S], bf16)          # Q^T
            kt_all = persist.tile([128, 2, S], bf16)          # K^T
            vp_all = persist.tile([128, ST_N, HG, DH + 1], bf16)  # V'
            w_all = persist.tile([128, 3, KD_N, CS], f32r)    # Wq,Wk,Wv 24KB/part

            # ---------------- phase B: X^T (X DMAs first — critical path) ---
            with (
                tc.tile_pool(name="xload", bufs=8) as xload,
                tc.tile_pool(name="wload", bufs=4) as wload,
                tc.tile_pool(name="tp", bufs=4, space="PSUM") as tp,
            ):
                x_tiles = []
                for st in range(ST_N):
                    x_tile = xload.tile([128, D_IN], f32)
                    nc.sync.dma_start(
                        out=x_tile, in_=X[st * 128:(st + 1) * 128, :]
                    )
                    x_tiles.append(x_tile)
                b_stage = consts.tile([1, 3, CS], f32)
                nc.sync.dma_start(out=b_stage[:, 0, :], in_=bq)
                nc.sync.dma_start(out=b_stage[:, 1, :], in_=bk)
                nc.sync.dma_start(out=b_stage[:, 2, :], in_=bv)
                b_rows = consts.tile([1, 3, CS], f32r)
                nc.vector.tensor_copy(b_rows, b_stage)

                for st in range(ST_N):
                    x_tile = x_tiles[st]
                    for kd in range(KD_N):
                        pt = tp.tile([128, 128], f32, space="PSUM")
                        nc.tensor.transpose(
                            pt, x_tile[:, kd * 128:(kd + 1) * 128], ident
                        )
                        dst = xt_all[:, kd, st * 128:(st + 1) * 128]
                        if (st * KD_N + kd) % 2 == 0:
                            nc.vector.tensor_copy(dst, pt)
                        else:
                            nc.scalar.copy(dst, pt)

                for wi, wap in enumerate((Wq, Wk, Wv)):
                    for kd in range(KD_N):
                        wst = wload.tile([128, CS], f32)
                        nc.sync.dma_start(
                            out=wst, in_=wap[kd * 128:(kd + 1) * 128, :]
                        )
                        dst = w_all[:, wi, kd, :]
                        if (wi * KD_N + kd) % 2 == 0:
                            nc.vector.tensor_copy(dst, wst)
                        else:
                            nc.scalar.copy(dst, wst)

                # ones column for the softmax-denominator trick
                for st in range(ST_N):
                    nc.vector.tensor_copy(vp_all[:, st, :, DH:DH + 1], ones4)

            # ---------------- phase C: projections ----------------
            with tc.tile_pool(name="pqk", bufs=3, space="PSUM") as pqk:
                for wi, dst_all in ((0, qt_all), (1, kt_all)):
                    for m in range(2):
                        for n_ in range(QC_N):
                            ps = pqk.tile([128, QCW], f32, space="PSUM")
                            for kd in range(KD_N):
                                nc.tensor.matmul(
                                    ps,
                                    lhsT=(w_all[:, wi, kd, m * 128:(m + 1) * 128]),
                                    rhs=(xt_all[:, kd, n_ * QCW:(n_ + 1) * QCW]),
                                    start=(kd == 0),
                                    stop=False,
                                )
                            nc.tensor.matmul(
                                ps,
                                lhsT=(b_rows[:, wi, m * 128:(m + 1) * 128]),
                                rhs=(ones_row),
                                start=False,
                                stop=True,
                            )
                            nc.scalar.copy(
                                dst_all[:, m, n_ * QCW:(n_ + 1) * QCW], ps
                            )

                for st in range(ST_N):
                    ps = pqk.tile([128, CS], f32, space="PSUM", tag="psv")
                    for kd in range(KD_N):
                        nc.tensor.matmul(
                            ps,
                            lhsT=(xt_all[:, kd, st * 128:(st + 1) * 128]),
                            rhs=(w_all[:, 2, kd, :]),
                            start=(kd == 0),
                            stop=False,
                        )
                    nc.tensor.matmul(
                        ps,
                        lhsT=(ones_row[:, 0:128]),
                        rhs=(b_rows[:, 2, :]),
                        start=False,
                        stop=True,
                    )
                    for j in range(HG):
                        nc.vector.tensor_copy(
                            vp_all[:, st, j, 0:DH], ps[:, j * DH:(j + 1) * DH]
                        )

            # ---------------- phase D: attention ----------------
            with (
                tc.tile_pool(name="sps", bufs=4, space="PSUM") as spsp,
                tc.tile_pool(name="acc", bufs=1, space="PSUM") as accp,
                tc.tile_pool(name="tp2", bufs=2, space="PSUM") as tp2,
                tc.tile_pool(name="et", bufs=3) as etp,
                tc.tile_pool(name="ot", bufs=2) as otp,
                tc.tile_pool(name="eps", bufs=4) as epp,
                tc.tile_pool(name="stage", bufs=2) as stagep,
            ):
                for qc in range(QC_N):
                    stage = stagep.tile([128, 4, CS], f32)
                    for mt in range(2):
                        # head pair (2*mt, 2*mt+1) lives on partitions 0-63 /
                        # 64-127 of kt_all/qt_all tile mt: their matmul1s run
                        # on independent 64-row PE tiles (T0/T8) concurrently.
                        jA, jB = 2 * mt, 2 * mt + 1
                        accA = accp.tile([DH + 1, QCW], f32, space="PSUM",
                                         tag="accA")
                        accB = accp.tile([DH + 1, QCW], f32, space="PSUM",
                                         tag="accB")
                        ets = []
                        # software-pipelined: kt's matmul1s issue before
                        # (kt-1)'s matmul2s so PE never stalls on ACT's exp
                        for kt in range(KT_N):
                            etA = etp.tile([128, QCW], bf16, tag="etA")
                            etB = etp.tile([128, QCW], bf16, tag="etB")
                            for po, et in ((0, etA), (64, etB)):
                                sps = spsp.tile([128, QCW], f32, space="PSUM")
                                nc.tensor.matmul(
                                    sps,
                                    lhsT=(kt_all[po:po + 64, mt, kt * 128:(kt + 1) * 128]),
                                    rhs=(qt_all[po:po + 64, mt, qc * QCW:(qc + 1) * QCW]),
                                    start=True,
                                    stop=True,
                                )
                                nc.scalar.activation(et, sps, Exp,
                                                     scale=1.0 / 4096.0)
                            ets.append((etA, etB))
                            if kt > 0:
                                pA, pB = ets[kt - 1]
                                nc.tensor.matmul(
                                    accA, lhsT=(vp_all[:, kt - 1, jA, :]),
                                    rhs=pA, start=(kt == 1), stop=False,
                                )
                                nc.tensor.matmul(
                                    accB, lhsT=(vp_all[:, kt - 1, jB, :]),
                                    rhs=pB, start=(kt == 1), stop=False,
                                )
                        pA, pB = ets[KT_N - 1]
                        nc.tensor.matmul(
                            accA, lhsT=(vp_all[:, KT_N - 1, jA, :]),
                            rhs=pA, start=False, stop=True,
                        )
                        nc.tensor.matmul(
                            accB, lhsT=(vp_all[:, KT_N - 1, jB, :]),
                            rhs=pB, start=False, stop=True,
                        )
                        # epilogue: transpose [65, 512] -> 4x [128, 65],
                        # divide by the denominator row, stage for DMA
                        for j, acc in ((jA, accA), (jB, accB)):
                            ot = otp.tile([DH + 1, QCW], f32)
                            nc.vector.tensor_copy(ot, acc)
                            for tj in range(4):
                                pt = tp2.tile([128, DH + 1], f32, space="PSUM")
                                nc.tensor.transpose(
                                    pt, ot[:, tj * 128:(tj + 1) * 128],
                                    ident[0:DH + 1, 0:DH + 1],
                                )
                                rcp = epp.tile([128, 1], f32)
                                nc.vector.reciprocal(rcp, pt[:, DH:DH + 1])
                                nc.vector.tensor_scalar_mul(
                                    stage[:, tj, j * DH:(j + 1) * DH],
                                    pt[:, 0:DH],
                                    rcp,
                                )
                    for tj in range(4):
                        nc.sync.dma_start(
                            out=O[qc * QCW + tj * 128: qc * QCW + (tj + 1) * 128, :],
                            in_=stage[:, tj, :],
                        )

    nc.compile()
    return nc


def _get_program():
    if "nc" not in _COMPILED:
        _COMPILED["nc"] = _build_program()
    return _COMPILED["nc"]


def make_in_maps(X, Wq, bq, Wk, bk, Wv, bv):
    in_maps = []
    for core in range(N_CORES):
        b, hg = core // HG, core % HG
        cs = slice(hg * CS, (hg + 1) * CS)
        in_maps.append({
            "X_s": np.ascontiguousarray(X[b]),
            "Wq_s": np.ascontiguousarray(Wq[:, cs]),
            "Wk_s": np.ascontiguousarray(Wk[:, cs]),
            "Wv_s": np.ascontiguousarray(Wv[:, cs]),
            "bq_s": np.ascontiguousarray(bq[cs]).reshape(1, CS),
            "bk_s": np.ascontiguousarray(bk[cs]).reshape(1, CS),
            "bv_s": np.ascontiguousarray(bv[cs]).reshape(1, CS),
        })
    return in_maps


def gather_output(results):
    out = np.zeros((2, S, D_OUT), np.float32)
    for core in range(N_CORES):
        b, hg = core // HG, core % HG
        out[b, :, hg * CS:(hg + 1) * CS] = results[core]["O_s"]
    return out


def run(X, Wq, bq, Wk, bk, Wv, bv, trace=False, tmpdir=None):
    from concourse import bass_utils

    nc = _get_program()
    in_maps = make_in_maps(X, Wq, bq, Wk, bk, Wv, bv)
    res = bass_utils.run_bass_kernel_spmd(
        nc, in_maps, core_ids=list(range(N_CORES)), trace=trace, tmpdir=tmpdir,
    )
    return gather_output(res.results), res


def kernel(X, Wq, bq, Wk, bk, Wv, bv):
    out, _ = run(
        np.asarray(X), np.asarray(Wq), np.asarray(bq), np.asarray(Wk),
        np.asarray(bk), np.asarray(Wv), np.asarray(bv),
    )
    return out


if __name__ == "__main__":
    rng = np.random.default_rng(0)
    inputs = {
        "X": rng.standard_normal((2, S, D_IN), dtype=np.float32),
        "Wq": rng.standard_normal((D_IN, D_OUT), dtype=np.float32) / 32,
        "bq": rng.standard_normal(D_OUT, dtype=np.float32) * 0.01,
        "Wk": rng.standard_normal((D_IN, D_OUT), dtype=np.float32) / 32,
        "bk": rng.standard_normal(D_OUT, dtype=np.float32) * 0.01,
        "Wv": rng.standard_normal((D_IN, D_OUT), dtype=np.float32) / 32,
        "bv": rng.standard_normal(D_OUT, dtype=np.float32) * 0.01,
    }
    out = kernel(**inputs)
    print("out", out.shape, out.dtype, np.abs(out).max())
